# revision 15
# baseline (speedup 1.0000x reference)
"""Trainium2 Bass kernel for nn_EquiConv2d (equirectangular deformable conv).

v4 — static per-band program, fully prefused x-interp:
  * off_y is longitude-invariant: each (tap k, row h) samples a fixed input
    row-pair (iy0, iy0+1) with constant y-fractions -> row-pair tiles F
    ([128 = 2rows x 64ch, 1024 = row duplicated for circular reads]).
  * off_x is longitude-invariant up to the 2*pi wrap: sampling along a row is
    a circular shift s0(k,h) plus a constant x-fraction fr.
  * EVERY genuine double tap is prefused on DVE/Act:
        z = win_a + c*win_b   (c = min(fr,1-fr)/max(..) <= 1)
    so each tap is ONE matmul [128-contraction, 512-free]: 9 full matmuls
    per row (v2 un-prefused half of them -> ~13 matmuls/row).
  * Seam zero-padding semantics handled without split matmuls:
      - prefused taps patch the single affected z column ([128,1] DVE op);
      - seam singles that fit a spare z-site become copy-builds (cc=0) with
        a memset patch;
      - leftover seam singles emit a 1-column NEGATED correction matmul into
        a separate PSUM tile at a static column, folded into the output row
        by a tiny DVE add (exact cancellation of the full-window matmul's
        seam-column contribution).
  * Staging ring = E (67 slots, no reuse/overwrite management).
  * fp32 oddities: tap (7,255) dead; tap (1,1) antipode handled by 3 extra
    data-driven matmul slots (band-0 section only).

Sharding: 8 cores = 2 batches x 4 bands of 64 output rows.
"""

import math

import numpy as np

# ----------------------------------------------------------------------------
# problem constants
B, C, H, W = 2, 64, 256, 512
O, KH, KW = 64, 3, 3
K = KH * KW
NCORES = 8
NROW = 64            # output rows per core
NZ = 5               # max z-build sites per row (excess doubles un-prefused)
NSING = 5            # max direct single-window matmul slots per row
NC = 4               # max 1-col correction matmul slots per row
NSLOT = NZ + NSING + NC
NSPEC = 3            # special (antipode) slots, accumulated into local row 1
PF = 3               # staging prefetch lead (rows)
NACT = 2             # z-builds per row whose mult half runs on Act
SKIP_TOL = 1e-4      # drop corner slots with |weight| below this
LTW = NSLOT * O      # stationary table columns per row

_CACHE = {}


# ----------------------------------------------------------------------------
# host-side geometry tables (must replicate reference fp32 semantics exactly)

def _compute_offsets_jax():
    """Bit-exact replica of reference.equi_offsets on jax CPU."""
    import jax
    import jax.numpy as jnp
    cpu = jax.devices("cpu")[0]
    with jax.default_device(cpu):
        dtype = jnp.float32
        pano_H, pano_W, kH, kW = H, W, KH, KW
        Kk = kH * kW
        u = jnp.arange(pano_W, dtype=dtype)
        v = jnp.arange(pano_H, dtype=dtype)
        phi = (u - pano_W / 2.0) / pano_W * (2.0 * math.pi)
        theta = -(v - pano_H / 2.0) / pano_H * math.pi
        cp, sp = jnp.cos(phi), jnp.sin(phi)
        z, one = jnp.zeros_like(cp), jnp.ones_like(cp)
        Ry = jnp.stack([jnp.stack([cp, z, sp], -1),
                        jnp.stack([z, one, z], -1),
                        jnp.stack([-sp, z, cp], -1)], -2)
        ct, st = jnp.cos(theta), jnp.sin(theta)
        zh, oh = jnp.zeros_like(ct), jnp.ones_like(ct)
        Rx = jnp.stack([jnp.stack([oh, zh, zh], -1),
                        jnp.stack([zh, ct, -st], -1),
                        jnp.stack([zh, st, ct], -1)], -2)
        ROT = jnp.einsum('wij,hjk->hwik', Ry, Rx)
        fov_w = kW * (2.0 * math.pi / pano_W)
        focal = (kW / 2.0) / math.tan(fov_w / 2.0)
        hg = (jnp.arange(kH, dtype=dtype)[:, None] + 0.5 - kH / 2.0)
        wg = (jnp.arange(kW, dtype=dtype)[None, :] + 0.5 - kW / 2.0)
        hg = jnp.broadcast_to(hg, (kH, kW)).reshape(Kk)
        wg = jnp.broadcast_to(wg, (kH, kW)).reshape(Kk)
        rays0 = jnp.stack([wg / focal, hg / focal, jnp.ones(Kk, dtype)], 0)
        rays0 = rays0 / jnp.linalg.norm(rays0, axis=0, keepdims=True)
        rays = jnp.einsum('hwik,kn->hwin', ROT, rays0)
        phi2 = jnp.arctan2(rays[..., 0, :], rays[..., 2, :])
        th2 = jnp.arcsin(jnp.clip(rays[..., 1, :], -1.0, 1.0))
        x = pano_W / (2.0 * math.pi) * phi2 + pano_W / 2.0
        y = pano_H / math.pi * th2 + pano_H / 2.0
        off_x = x - (wg[None, None, :] + u[None, :, None])
        off_y = y - (hg[None, None, :] + v[:, None, None])
        return (np.asarray(jnp.transpose(off_y, (2, 0, 1))),
                np.asarray(jnp.transpose(off_x, (2, 0, 1))))


def _build_tap_tables():
    off_y, off_x = _compute_offsets_jax()
    ky = np.repeat(np.arange(KH), KW).astype(np.float32)
    kx = np.tile(np.arange(KW), KH).astype(np.float32)
    base_x = (np.arange(W, dtype=np.float32) - np.float32(1))
    base_y = (np.arange(H, dtype=np.float32) - np.float32(1))
    px = (base_x[None, None, :] + kx[:, None, None] + off_x).astype(np.float32)
    py = (base_y[None, :, None] + ky[:, None, None] + off_y).astype(np.float32)
    pyc = py[:, :, 0]
    assert np.all(py == pyc[:, :, None]), "off_y not longitude-invariant"

    iy0 = np.floor(pyc).astype(np.int64)
    wy1 = (pyc - np.floor(pyc)).astype(np.float64)
    v0 = (iy0 >= 0) & (iy0 < H)
    v1 = (iy0 + 1 >= 0) & (iy0 + 1 < H)
    cy0 = np.where(v0, 1.0 - wy1, 0.0)
    cy1 = np.where(v1, wy1, 0.0)

    Draw = np.mod((px.astype(np.float64) - np.arange(W)[None, None, :]), 512.0)
    ang = Draw / 512.0 * 2 * np.pi
    mean = np.mod(np.angle(np.exp(1j * ang).mean(axis=2)) / (2 * np.pi) * 512.0,
                  512.0)
    resid = np.mod(Draw - mean[:, :, None] + 256.0, 512.0) - 256.0
    D = mean + np.median(resid, axis=2)
    s0 = np.mod(np.floor(D), 512).astype(np.int64)
    frac = D - np.floor(D)

    special = np.zeros((K, H), dtype=bool)
    special[1, 1] = True
    dead = (cy0 == 0.0) & (cy1 == 0.0)

    Ddev = np.abs(np.mod(Draw - D[:, :, None] + 256.0, 512.0) - 256.0)
    dev = Ddev.max(axis=2)
    bad = (dev > 5e-4) & ~special & ~dead
    assert not bad.any(), f"unrepresentable taps: {np.argwhere(bad)}"

    def ref_coefs(p):
        x0 = math.floor(p)
        fr = p - x0
        out = {}
        for ix, wt in ((x0, 1.0 - fr), (x0 + 1, fr)):
            if 0 <= ix < W and wt != 0.0:
                out[ix] = out.get(ix, 0.0) + wt
        return out

    # seam variant selection: decided by the exact fp32 px at the wrap column
    slot0_useG = np.zeros((K, H), dtype=bool)
    slot1_useF = np.zeros((K, H), dtype=bool)
    for k in range(K):
        for h in range(H):
            if special[k, h] or dead[k, h]:
                continue
            s = int(s0[k, h]); fr = frac[k, h]
            if s >= 1:
                w0 = (512 - s) % 512
                rc = ref_coefs(float(px[k, h, w0]))
                slot0_useG[k, h] = (abs(rc.get(0, 0.0))
                                    < abs(rc.get(0, 0.0) - (1 - fr)))
            w1 = (511 - s) % 512
            rc = ref_coefs(float(px[k, h, w1]))
            slot1_useF[k, h] = (abs(rc.get(0, 0.0) - fr)
                                < abs(rc.get(0, 0.0)))

    # special tap (1,1): per-column coefficients on F offsets 255..257
    pxs = px[1, 1, :].astype(np.float64)
    Gam = np.zeros((3, W), dtype=np.float64)
    for w in range(W):
        p = pxs[w]
        x0 = math.floor(p)
        fr = p - x0
        for ix, wt in ((x0, 1.0 - fr), (x0 + 1, fr)):
            if 0 <= ix < W and wt != 0.0:
                found = False
                for jj in range(3):
                    if (255 + jj + w) % 512 == ix % 512:
                        Gam[jj, w] += wt
                        found = True
                        break
                assert found, (w, p, ix)

    return dict(iy0=iy0, cy0=cy0, cy1=cy1, s0=s0, frac=frac,
                slot0_useG=slot0_useG, slot1_useF=slot1_useF,
                special=special, dead=dead, Gam=Gam)


# ----------------------------------------------------------------------------
# SPMD schedule (events = staged row-pairs; slot index == event index)

def _build_schedule(tt):
    blocks = []
    for blk in range(4):
        h0 = blk * NROW
        ev_of, events, first_use = {}, [], []
        need = np.zeros((NROW, K), np.int64)
        for lh in range(NROW):
            for k in range(K):
                r = int(np.clip(tt['iy0'][k, h0 + lh], 0, 255))
                if r not in ev_of:
                    ev_of[r] = len(events)
                    events.append(r)
                    first_use.append(lh)
                need[lh, k] = ev_of[r]
        blocks.append(dict(events=events, first_use=first_use, need=need))

    E = max(len(b['events']) for b in blocks)
    for b in blocks:
        while len(b['events']) < E:
            b['events'].append(b['events'][-1])

    # uniform per-row staging targets (union over bands)
    hi = np.zeros(NROW, np.int64)
    for b in blocks:
        need = b['need']
        for lh in range(NROW):
            hi[lh] = max(hi[lh], need[lh].max())
    avail = hi + 1
    tgt = [int(avail[min(lh + PF, NROW - 1)]) for lh in range(NROW)]
    tgt[-1] = E
    return blocks, E, tgt


# ----------------------------------------------------------------------------
# per-(band,row) slot packing
#
#   z-sites (<=NZ)  : build z = win_a + cc*win_b (+1 patch), full matmul on z
#   singles (<=NSING): full matmul on a static buf window
#   corr (<=NC)     : 1-col negated matmul into pscorr + DVE fold into out

def _pack_rows(tt, blocks):
    packs = []
    for blk in range(4):
        need = blocks[blk]['need']
        rows = []
        for lh in range(NROW):
            h = blk * NROW + lh
            doubles, singles = [], []
            for k in range(K):
                if tt['dead'][k, h] or tt['special'][k, h]:
                    continue
                ev = int(need[lh, k])
                s = int(tt['s0'][k, h])
                fr = float(tt['frac'][k, h])
                c0 = float(tt['cy0'][k, h])
                c1 = float(tt['cy1'][k, h])
                f0a, f0b = s, s + 1
                a_fp = bool(tt['slot0_useG'][k, h]) and s >= 1 and f0a >= 1
                b_fp = (not bool(tt['slot1_useF'][k, h])) and f0b >= 1
                e0 = 1.0 - fr >= SKIP_TOL
                e1 = fr >= SKIP_TOL
                if e0 and e1:
                    if fr <= 0.5:
                        aw, cc = 1.0 - fr, fr / (1.0 - fr)
                        wa, wb, wa_fp, wb_fp = f0a, f0b, a_fp, b_fp
                    else:
                        aw, cc = fr, (1.0 - fr) / fr
                        wa, wb, wa_fp, wb_fp = f0b, f0a, b_fp, a_fp
                    assert not (wa_fp and wb_fp), (blk, lh, k)
                    if wa_fp:
                        # win_a reads seam-zero at pa: z[:,pa] = cc*winb[pa]
                        patch = (512 - wa, 'a', cc)
                    elif wb_fp:
                        # win_b reads seam-zero at pb: z[:,pb] = wina[pb]
                        patch = (512 - wb, 'b', 1.0)
                    else:
                        patch = None
                    doubles.append(dict(k=k, ev=ev, wa=wa, wb=wb, cc=cc,
                                        patch=patch, s0=c0 * aw, s1=c1 * aw))
                elif e0 or e1:
                    wt, f0, fp = ((1.0 - fr), f0a, a_fp) if e0 \
                        else (fr, f0b, b_fp)
                    ws = 512 - f0 if (fp and 0 <= 512 - f0 <= 511) else None
                    singles.append(dict(k=k, ev=ev, f0=f0, ws=ws,
                                        s0=c0 * wt, s1=c1 * wt))

            # un-prefuse excess doubles (prefer patch-less ones) into
            # 2 corner singles each
            doubles.sort(key=lambda d: d['patch'] is None)
            while len(doubles) > NZ:
                d = doubles.pop()
                pc, typ = (d['patch'][0], d['patch'][1]) \
                    if d['patch'] is not None else (None, None)
                singles.append(dict(k=d['k'], ev=d['ev'], f0=d['wa'],
                                    ws=(pc if typ == 'a' else None),
                                    s0=d['s0'], s1=d['s1']))
                singles.append(dict(k=d['k'], ev=d['ev'], f0=d['wb'],
                                    ws=(pc if typ == 'b' else None),
                                    s0=d['s0'] * d['cc'],
                                    s1=d['s1'] * d['cc']))
            zsites = list(doubles)
            plain, corr = [], []
            seam = [sg for sg in singles if sg['ws'] is not None]
            nonseam = [sg for sg in singles if sg['ws'] is None]
            for sg in seam:
                if len(zsites) < NZ:
                    # seam single as z-site: cc=0 copy-build, memset patch
                    zsites.append(dict(k=sg['k'], ev=sg['ev'], wa=sg['f0'],
                                       wb=sg['f0'], cc=0.0,
                                       patch=(sg['ws'], 'z', 0.0),
                                       s0=sg['s0'], s1=sg['s1']))
                else:
                    plain.append(sg)
                    corr.append(dict(k=sg['k'], ev=sg['ev'],
                                     col=sg['f0'] + sg['ws'], fo=sg['ws'],
                                     s0=sg['s0'], s1=sg['s1']))
            for sg in nonseam:
                if len(plain) < NSING:
                    plain.append(sg)
                elif len(zsites) < NZ:
                    zsites.append(dict(k=sg['k'], ev=sg['ev'], wa=sg['f0'],
                                       wb=sg['f0'], cc=0.0, patch=None,
                                       s0=sg['s0'], s1=sg['s1']))
                else:
                    raise AssertionError((blk, lh, "slot overflow"))
            assert len(plain) <= NSING, (blk, lh, len(plain))
            assert len(corr) <= NC, (blk, lh, len(corr))
            assert len(plain) >= 1, (blk, lh)   # group-open matmul
            rows.append(dict(z=zsites, sing=plain, corr=corr))
        packs.append(rows)
    return packs


# ----------------------------------------------------------------------------
# device program

def _emit_kernel(tc, aps, E, packs, tgt, spec_ev):
    import concourse.mybir as mybir
    nc = tc.nc
    f16 = mybir.dt.float16
    f32 = mybir.dt.float32
    AL = mybir.AluOpType
    ID = mybir.ActivationFunctionType.Identity

    with tc.tile_pool(name="bigp", bufs=1) as bigp, \
         tc.tile_pool(name="ltp", bufs=6) as ltp, \
         tc.tile_pool(name="zp", bufs=2) as zp, \
         tc.tile_pool(name="zmp", bufs=2) as zmp, \
         tc.tile_pool(name="spzp", bufs=1) as spzp, \
         tc.tile_pool(name="psp", bufs=6, space="PSUM") as psp, \
         tc.tile_pool(name="outp", bufs=10) as outp:

        buf = bigp.tile([128, E * 1024], f16)
        coeft = bigp.tile([128, NSPEC * W], f16)
        biast = bigp.tile([O, 1], f32)
        ltst = bigp.tile([128, NSPEC * O], f16)

        blkv = nc.values_load(aps['blkid'][0:1, 0:1],
                              min_val=0, max_val=3,
                              skip_runtime_bounds_check=True)

        def stage(e):
            base = e * 1024
            src = aps['xb'][e].rearrange("p c w -> (p c) w")
            nc.sync.dma_start(buf[:, base:base + W], src)
            nc.sync.dma_start(buf[:, base + W:base + 2 * W], src)

        def emit_branch(r, ps, zts, zms, ltt, szts, is_spec):
            nmm = len(r['sing']) + len(r['z']) + (NSPEC if is_spec else 0)
            mi = 0
            # first single opens the accumulation group (full width)
            sg = r['sing'][0]
            base = sg['ev'] * 1024
            nc.tensor.matmul(ps, ltt[:, NZ * O:(NZ + 1) * O],
                             buf[:, base + sg['f0']:base + sg['f0'] + W],
                             start=True, stop=(nmm == 1))
            mi = 1
            # 1-col corrections accumulate into the same group
            for j, cr in enumerate(r['corr']):
                si = NZ + NSING + j
                col = cr['ev'] * 1024 + cr['col']
                fo = cr['fo']
                nc.tensor.matmul(ps[:, fo:fo + 1],
                                 ltt[:, si * O:(si + 1) * O],
                                 buf[:, col:col + 1],
                                 start=False, stop=False)
            for j, sg in enumerate(r['sing'][1:], start=1):
                si = NZ + j
                base = sg['ev'] * 1024
                nc.tensor.matmul(ps, ltt[:, si * O:(si + 1) * O],
                                 buf[:, base + sg['f0']:base + sg['f0'] + W],
                                 start=False, stop=(mi == nmm - 1))
                mi += 1
            # z-builds: ts (mult) + tt (add) + patch
            for j, d in enumerate(r['z']):
                base = d['ev'] * 1024
                wina = buf[:, base + d['wa']:base + d['wa'] + W]
                winb = buf[:, base + d['wb']:base + d['wb'] + W]
                if j < NACT:
                    nc.scalar.activation(zms[j], winb, ID, bias=0.0,
                                         scale=float(d['cc']))
                else:
                    nc.vector.tensor_scalar(zms[j], winb, float(d['cc']),
                                            None, AL.mult)
                nc.vector.tensor_tensor(zts[j], zms[j], wina, AL.add)
                if d['patch'] is not None:
                    pc, typ, psc = d['patch']
                    if typ == 'a':
                        nc.vector.tensor_scalar(zts[j][:, pc:pc + 1],
                                                winb[:, pc:pc + 1],
                                                float(psc), None, AL.mult)
                    elif typ == 'b':
                        nc.vector.tensor_copy(zts[j][:, pc:pc + 1],
                                              wina[:, pc:pc + 1])
                    else:
                        nc.vector.memset(zts[j][:, pc:pc + 1], 0.0)
            for j in range(len(r['z'])):
                nc.tensor.matmul(ps, ltt[:, j * O:(j + 1) * O], zts[j],
                                 start=False, stop=(mi == nmm - 1))
                mi += 1
            # antipode specials (band 0, local row 1)
            if is_spec:
                sbase = spec_ev * 1024
                for jj in range(NSPEC):
                    nc.vector.tensor_tensor(
                        szts[jj],
                        buf[:, sbase + 255 + jj:sbase + 255 + jj + W],
                        coeft[:, jj * W:(jj + 1) * W], AL.mult)
                    nc.tensor.matmul(ps, ltst[:, jj * O:(jj + 1) * O],
                                     szts[jj],
                                     start=False, stop=(mi == nmm - 1))
                    mi += 1

        staged = 0
        for lh in range(NROW):
            while staged < tgt[lh]:
                stage(staged)
                staged += 1
            ltt = ltp.tile([128, LTW], f16, tag="ltt")
            nc.sync.dma_start(ltt, aps['lt'][lh])
            if lh == 0:
                nc.sync.dma_start(biast, aps['biasd'])
                nc.sync.dma_start(ltst, aps['lts'])
                nc.sync.dma_start(coeft, aps['coefr'])
            ps = psp.tile([O, W], f32, tag="ps")
            nz = max(len(packs[blk][lh]['z']) for blk in range(4))
            zts = [zp.tile([128, W], f16, tag=f"z{j}", name=f"zt{j}")
                   for j in range(nz)]
            zms = [zmp.tile([128, W], f16, tag=f"zm{j}", name=f"zm{j}")
                   for j in range(nz)]
            szts = [spzp.tile([128, W], f16, tag=f"spz{jj}", name=f"szt{jj}")
                    for jj in range(NSPEC)] if lh == 1 else None
            for blk in range(4):
                with tc.If(blkv == blk):
                    emit_branch(packs[blk][lh], ps, zts, zms, ltt, szts,
                                is_spec=(blk == 0 and lh == 1))
            ot = outp.tile([O, W], f16, tag="out")
            nc.scalar.activation(ot, ps, ID, bias=biast, scale=1.0)
            nc.sync.dma_start(aps['out'][lh], ot)


def _get_compiled():
    """Build tables, schedule, and the Bass program once."""
    if 'prog' in _CACHE:
        return _CACHE['prog']
    import concourse.mybir as mybir
    import concourse.tile as tile
    from concourse import bacc

    tt = _build_tap_tables()
    blocks, E, tgt = _build_schedule(tt)
    packs = _pack_rows(tt, blocks)
    spec_ev = int(blocks[0]['need'][1, 1])

    f16 = mybir.dt.float16
    f32 = mybir.dt.float32
    nc = bacc.Bacc("TRN2", target_bir_lowering=False, debug=False,
                   num_devices=NCORES)
    aps = {
        'xb': nc.dram_tensor("xb", [E, 2, C, W], f16,
                             kind="ExternalInput").ap(),
        'lt': nc.dram_tensor("lt", [NROW, 128, LTW], f16,
                             kind="ExternalInput").ap(),
        'lts': nc.dram_tensor("lts", [128, NSPEC * O], f16,
                              kind="ExternalInput").ap(),
        'blkid': nc.dram_tensor("blkid", [1, 1], mybir.dt.int32,
                                kind="ExternalInput").ap(),
        'coefr': nc.dram_tensor("coefr", [128, NSPEC * W], f16,
                                kind="ExternalInput").ap(),
        'biasd': nc.dram_tensor("biasd", [O, 1], f32,
                                kind="ExternalInput").ap(),
        'out': nc.dram_tensor("out", [NROW, O, W], f16,
                              kind="ExternalOutput").ap(),
    }
    with tile.TileContext(nc) as tc:
        _emit_kernel(tc, aps, E, packs, tgt, spec_ev)
    nc.finalize()

    _CACHE['prog'] = (nc, tt, blocks, E, packs)
    return _CACHE['prog']


def _core_inputs(x, weight, bias, tt, blocks, E, packs):
    """Assemble per-core in_maps. Core c = batch (c // 4), band (c % 4)."""
    w3 = weight.reshape(O, C, K).astype(np.float64)
    w2d = np.empty((128, K, O), np.float64)
    w2d[:C] = w3.transpose(1, 2, 0)
    w2d[C:] = w3.transpose(1, 2, 0)
    biasd = np.ascontiguousarray(bias.reshape(O, 1).astype(np.float32))

    lts_on = np.zeros((128, NSPEC * O), np.float16)
    for jj in range(NSPEC):
        lts_on[:C, jj * O:(jj + 1) * O] = w2d[:C, 1, :].astype(np.float16)
    lts_off = np.zeros((128, NSPEC * O), np.float16)

    Gam = tt['Gam'].astype(np.float16)
    coef_on = np.ascontiguousarray(
        np.broadcast_to(Gam[:, None, :], (NSPEC, 128, W))
        .transpose(1, 0, 2).reshape(128, NSPEC * W))
    coef_off = np.zeros((128, NSPEC * W), np.float16)

    lt_blk = []
    for blk in range(4):
        ltv = np.zeros((NROW, 128, LTW), np.float16)
        for lh in range(NROW):
            r = packs[blk][lh]
            for j, d in enumerate(r['z']):
                blkw = np.empty((128, O), np.float64)
                blkw[:64] = w2d[:64, d['k'], :] * d['s0']
                blkw[64:] = w2d[64:, d['k'], :] * d['s1']
                ltv[lh, :, j * O:(j + 1) * O] = blkw.astype(np.float16)
            for j, sg in enumerate(r['sing']):
                si = NZ + j
                blkw = np.empty((128, O), np.float64)
                blkw[:64] = w2d[:64, sg['k'], :] * sg['s0']
                blkw[64:] = w2d[64:, sg['k'], :] * sg['s1']
                ltv[lh, :, si * O:(si + 1) * O] = blkw.astype(np.float16)
            for j, cr in enumerate(r['corr']):
                si = NZ + NSING + j
                blkw = np.empty((128, O), np.float64)
                blkw[:64] = w2d[:64, cr['k'], :] * -cr['s0']
                blkw[64:] = w2d[64:, cr['k'], :] * -cr['s1']
                ltv[lh, :, si * O:(si + 1) * O] = blkw.astype(np.float16)
        lt_blk.append(np.ascontiguousarray(ltv))

    in_maps = []
    for cid in range(NCORES):
        b, blk = cid // 4, cid % 4
        xz = np.concatenate([x[b], np.zeros((C, 1, W), x.dtype)], axis=1)
        xz = xz.astype(np.float16)
        rows = np.asarray(blocks[blk]['events'], np.int64)
        pair_idx = np.stack([rows, rows + 1], axis=1)       # [E, 2]
        xbv = xz[:, pair_idx, :]                            # [C, E, 2, W]
        xbv = np.ascontiguousarray(xbv.transpose(1, 2, 0, 3))  # [E,2,C,W]
        in_maps.append({
            'xb': xbv,
            'lt': lt_blk[blk],
            'lts': lts_on if blk == 0 else lts_off,
            'blkid': np.array([[blk]], np.int32),
            'coefr': coef_on if blk == 0 else coef_off,
            'biasd': biasd,
        })
    return in_maps


def kernel(x, weight, bias):
    from concourse.bass_utils import run_bass_kernel_spmd
    x = np.asarray(x, dtype=np.float32)
    weight = np.asarray(weight, dtype=np.float32)
    bias = np.asarray(bias, dtype=np.float32)

    nc, tt, blocks, E, packs = _get_compiled()
    in_maps = _core_inputs(x, weight, bias, tt, blocks, E, packs)
    res = run_bass_kernel_spmd(nc, in_maps, core_ids=list(range(NCORES)))

    out = np.empty((B, O, H, W), np.float32)
    for cid in range(NCORES):
        b, blk = cid // 4, cid % 4
        oc = res.results[cid]['out'].astype(np.float32)     # [NROW, O, W]
        out[b, :, blk * NROW:(blk + 1) * NROW, :] = oc.transpose(1, 0, 2)
    return out


# revision 16
# speedup vs baseline: 1.7663x; 1.7663x over previous
"""Trainium2 Bass kernel for nn_EquiConv2d (equirectangular deformable conv).

v4 — static per-band program, fully prefused x-interp:
  * off_y is longitude-invariant: each (tap k, row h) samples a fixed input
    row-pair (iy0, iy0+1) with constant y-fractions -> row-pair tiles F
    ([128 = 2rows x 64ch, 1024 = row duplicated for circular reads]).
  * off_x is longitude-invariant up to the 2*pi wrap: sampling along a row is
    a circular shift s0(k,h) plus a constant x-fraction fr.
  * EVERY genuine double tap is prefused on DVE/Act:
        z = win_a + c*win_b   (c = min(fr,1-fr)/max(..) <= 1)
    so each tap is ONE matmul [128-contraction, 512-free]: 9 full matmuls
    per row (v2 un-prefused half of them -> ~13 matmuls/row).
  * Seam zero-padding semantics handled without split matmuls:
      - prefused taps patch the single affected z column ([128,1] DVE op);
      - seam singles that fit a spare z-site become copy-builds (cc=0) with
        a memset patch;
      - leftover seam singles emit a 1-column NEGATED correction matmul into
        a separate PSUM tile at a static column, folded into the output row
        by a tiny DVE add (exact cancellation of the full-window matmul's
        seam-column contribution).
  * Staging ring = E (67 slots, no reuse/overwrite management).
  * fp32 oddities: tap (7,255) dead; tap (1,1) antipode handled by 3 extra
    data-driven matmul slots (band-0 section only).

Sharding: 8 cores = 2 batches x 4 bands of 64 output rows.
"""

import math

import numpy as np

# ----------------------------------------------------------------------------
# problem constants
B, C, H, W = 2, 64, 256, 512
O, KH, KW = 64, 3, 3
K = KH * KW
NCORES = 8
NROW = 64            # output rows per core
NZ = 5               # max z-build sites per row (excess doubles un-prefused)
NSING = 5            # max direct single-window matmul slots per row
NC = 4               # max 1-col correction matmul slots per row
NSLOT = NZ + NSING + NC
NSPEC = 3            # special (antipode) slots, accumulated into local row 1
PF = 3               # staging prefetch lead (rows)
NACT = 2             # z-builds per row whose mult half runs on Act
SKIP_TOL = 1e-4      # drop corner slots with |weight| below this
LTW = NSLOT * O      # stationary table columns per row

_CACHE = {}


# ----------------------------------------------------------------------------
# host-side geometry tables (must replicate reference fp32 semantics exactly)

def _compute_offsets_jax():
    """Bit-exact replica of reference.equi_offsets on jax CPU."""
    import jax
    import jax.numpy as jnp
    cpu = jax.devices("cpu")[0]
    with jax.default_device(cpu):
        dtype = jnp.float32
        pano_H, pano_W, kH, kW = H, W, KH, KW
        Kk = kH * kW
        u = jnp.arange(pano_W, dtype=dtype)
        v = jnp.arange(pano_H, dtype=dtype)
        phi = (u - pano_W / 2.0) / pano_W * (2.0 * math.pi)
        theta = -(v - pano_H / 2.0) / pano_H * math.pi
        cp, sp = jnp.cos(phi), jnp.sin(phi)
        z, one = jnp.zeros_like(cp), jnp.ones_like(cp)
        Ry = jnp.stack([jnp.stack([cp, z, sp], -1),
                        jnp.stack([z, one, z], -1),
                        jnp.stack([-sp, z, cp], -1)], -2)
        ct, st = jnp.cos(theta), jnp.sin(theta)
        zh, oh = jnp.zeros_like(ct), jnp.ones_like(ct)
        Rx = jnp.stack([jnp.stack([oh, zh, zh], -1),
                        jnp.stack([zh, ct, -st], -1),
                        jnp.stack([zh, st, ct], -1)], -2)
        ROT = jnp.einsum('wij,hjk->hwik', Ry, Rx)
        fov_w = kW * (2.0 * math.pi / pano_W)
        focal = (kW / 2.0) / math.tan(fov_w / 2.0)
        hg = (jnp.arange(kH, dtype=dtype)[:, None] + 0.5 - kH / 2.0)
        wg = (jnp.arange(kW, dtype=dtype)[None, :] + 0.5 - kW / 2.0)
        hg = jnp.broadcast_to(hg, (kH, kW)).reshape(Kk)
        wg = jnp.broadcast_to(wg, (kH, kW)).reshape(Kk)
        rays0 = jnp.stack([wg / focal, hg / focal, jnp.ones(Kk, dtype)], 0)
        rays0 = rays0 / jnp.linalg.norm(rays0, axis=0, keepdims=True)
        rays = jnp.einsum('hwik,kn->hwin', ROT, rays0)
        phi2 = jnp.arctan2(rays[..., 0, :], rays[..., 2, :])
        th2 = jnp.arcsin(jnp.clip(rays[..., 1, :], -1.0, 1.0))
        x = pano_W / (2.0 * math.pi) * phi2 + pano_W / 2.0
        y = pano_H / math.pi * th2 + pano_H / 2.0
        off_x = x - (wg[None, None, :] + u[None, :, None])
        off_y = y - (hg[None, None, :] + v[:, None, None])
        return (np.asarray(jnp.transpose(off_y, (2, 0, 1))),
                np.asarray(jnp.transpose(off_x, (2, 0, 1))))


def _build_tap_tables():
    off_y, off_x = _compute_offsets_jax()
    ky = np.repeat(np.arange(KH), KW).astype(np.float32)
    kx = np.tile(np.arange(KW), KH).astype(np.float32)
    base_x = (np.arange(W, dtype=np.float32) - np.float32(1))
    base_y = (np.arange(H, dtype=np.float32) - np.float32(1))
    px = (base_x[None, None, :] + kx[:, None, None] + off_x).astype(np.float32)
    py = (base_y[None, :, None] + ky[:, None, None] + off_y).astype(np.float32)
    pyc = py[:, :, 0]
    assert np.all(py == pyc[:, :, None]), "off_y not longitude-invariant"

    iy0 = np.floor(pyc).astype(np.int64)
    wy1 = (pyc - np.floor(pyc)).astype(np.float64)
    v0 = (iy0 >= 0) & (iy0 < H)
    v1 = (iy0 + 1 >= 0) & (iy0 + 1 < H)
    cy0 = np.where(v0, 1.0 - wy1, 0.0)
    cy1 = np.where(v1, wy1, 0.0)

    Draw = np.mod((px.astype(np.float64) - np.arange(W)[None, None, :]), 512.0)
    ang = Draw / 512.0 * 2 * np.pi
    mean = np.mod(np.angle(np.exp(1j * ang).mean(axis=2)) / (2 * np.pi) * 512.0,
                  512.0)
    resid = np.mod(Draw - mean[:, :, None] + 256.0, 512.0) - 256.0
    D = mean + np.median(resid, axis=2)
    s0 = np.mod(np.floor(D), 512).astype(np.int64)
    frac = D - np.floor(D)

    special = np.zeros((K, H), dtype=bool)
    special[1, 1] = True
    dead = (cy0 == 0.0) & (cy1 == 0.0)

    Ddev = np.abs(np.mod(Draw - D[:, :, None] + 256.0, 512.0) - 256.0)
    dev = Ddev.max(axis=2)
    bad = (dev > 5e-4) & ~special & ~dead
    assert not bad.any(), f"unrepresentable taps: {np.argwhere(bad)}"

    def ref_coefs(p):
        x0 = math.floor(p)
        fr = p - x0
        out = {}
        for ix, wt in ((x0, 1.0 - fr), (x0 + 1, fr)):
            if 0 <= ix < W and wt != 0.0:
                out[ix] = out.get(ix, 0.0) + wt
        return out

    # seam variant selection: decided by the exact fp32 px at the wrap column
    slot0_useG = np.zeros((K, H), dtype=bool)
    slot1_useF = np.zeros((K, H), dtype=bool)
    for k in range(K):
        for h in range(H):
            if special[k, h] or dead[k, h]:
                continue
            s = int(s0[k, h]); fr = frac[k, h]
            if s >= 1:
                w0 = (512 - s) % 512
                rc = ref_coefs(float(px[k, h, w0]))
                slot0_useG[k, h] = (abs(rc.get(0, 0.0))
                                    < abs(rc.get(0, 0.0) - (1 - fr)))
            w1 = (511 - s) % 512
            rc = ref_coefs(float(px[k, h, w1]))
            slot1_useF[k, h] = (abs(rc.get(0, 0.0) - fr)
                                < abs(rc.get(0, 0.0)))

    # special tap (1,1): per-column coefficients on F offsets 255..257
    pxs = px[1, 1, :].astype(np.float64)
    Gam = np.zeros((3, W), dtype=np.float64)
    for w in range(W):
        p = pxs[w]
        x0 = math.floor(p)
        fr = p - x0
        for ix, wt in ((x0, 1.0 - fr), (x0 + 1, fr)):
            if 0 <= ix < W and wt != 0.0:
                found = False
                for jj in range(3):
                    if (255 + jj + w) % 512 == ix % 512:
                        Gam[jj, w] += wt
                        found = True
                        break
                assert found, (w, p, ix)

    return dict(iy0=iy0, cy0=cy0, cy1=cy1, s0=s0, frac=frac,
                slot0_useG=slot0_useG, slot1_useF=slot1_useF,
                special=special, dead=dead, Gam=Gam)


# ----------------------------------------------------------------------------
# SPMD schedule (events = staged row-pairs; slot index == event index)

def _build_schedule(tt):
    blocks = []
    for blk in range(4):
        h0 = blk * NROW
        ev_of, events, first_use = {}, [], []
        need = np.zeros((NROW, K), np.int64)
        for lh in range(NROW):
            for k in range(K):
                r = int(np.clip(tt['iy0'][k, h0 + lh], 0, 255))
                if r not in ev_of:
                    ev_of[r] = len(events)
                    events.append(r)
                    first_use.append(lh)
                need[lh, k] = ev_of[r]
        blocks.append(dict(events=events, first_use=first_use, need=need))

    E = max(len(b['events']) for b in blocks)
    for b in blocks:
        while len(b['events']) < E:
            b['events'].append(b['events'][-1])

    # uniform per-row staging targets (union over bands)
    hi = np.zeros(NROW, np.int64)
    for b in blocks:
        need = b['need']
        for lh in range(NROW):
            hi[lh] = max(hi[lh], need[lh].max())
    avail = hi + 1
    tgt = [int(avail[min(lh + PF, NROW - 1)]) for lh in range(NROW)]
    tgt[-1] = E
    return blocks, E, tgt


# ----------------------------------------------------------------------------
# per-(band,row) slot packing
#
#   z-sites (<=NZ)  : build z = win_a + cc*win_b (+1 patch), full matmul on z
#   singles (<=NSING): full matmul on a static buf window
#   corr (<=NC)     : 1-col negated matmul into pscorr + DVE fold into out

def _pack_rows(tt, blocks):
    packs = []
    for blk in range(4):
        need = blocks[blk]['need']
        rows = []
        for lh in range(NROW):
            h = blk * NROW + lh
            doubles, singles = [], []
            for k in range(K):
                if tt['dead'][k, h] or tt['special'][k, h]:
                    continue
                ev = int(need[lh, k])
                s = int(tt['s0'][k, h])
                fr = float(tt['frac'][k, h])
                c0 = float(tt['cy0'][k, h])
                c1 = float(tt['cy1'][k, h])
                f0a, f0b = s, s + 1
                a_fp = bool(tt['slot0_useG'][k, h]) and s >= 1 and f0a >= 1
                b_fp = (not bool(tt['slot1_useF'][k, h])) and f0b >= 1
                e0 = 1.0 - fr >= SKIP_TOL
                e1 = fr >= SKIP_TOL
                if e0 and e1:
                    if fr <= 0.5:
                        aw, cc = 1.0 - fr, fr / (1.0 - fr)
                        wa, wb, wa_fp, wb_fp = f0a, f0b, a_fp, b_fp
                    else:
                        aw, cc = fr, (1.0 - fr) / fr
                        wa, wb, wa_fp, wb_fp = f0b, f0a, b_fp, a_fp
                    assert not (wa_fp and wb_fp), (blk, lh, k)
                    if wa_fp:
                        # win_a reads seam-zero at pa: z[:,pa] = cc*winb[pa]
                        patch = (512 - wa, 'a', cc)
                    elif wb_fp:
                        # win_b reads seam-zero at pb: z[:,pb] = wina[pb]
                        patch = (512 - wb, 'b', 1.0)
                    else:
                        patch = None
                    doubles.append(dict(k=k, ev=ev, wa=wa, wb=wb, cc=cc,
                                        patch=patch, s0=c0 * aw, s1=c1 * aw))
                elif e0 or e1:
                    wt, f0, fp = ((1.0 - fr), f0a, a_fp) if e0 \
                        else (fr, f0b, b_fp)
                    ws = 512 - f0 if (fp and 0 <= 512 - f0 <= 511) else None
                    singles.append(dict(k=k, ev=ev, f0=f0, ws=ws,
                                        s0=c0 * wt, s1=c1 * wt))

            # un-prefuse excess doubles (prefer patch-less ones) into
            # 2 corner singles each
            doubles.sort(key=lambda d: d['patch'] is None)
            while len(doubles) > NZ:
                d = doubles.pop()
                pc, typ = (d['patch'][0], d['patch'][1]) \
                    if d['patch'] is not None else (None, None)
                singles.append(dict(k=d['k'], ev=d['ev'], f0=d['wa'],
                                    ws=(pc if typ == 'a' else None),
                                    s0=d['s0'], s1=d['s1']))
                singles.append(dict(k=d['k'], ev=d['ev'], f0=d['wb'],
                                    ws=(pc if typ == 'b' else None),
                                    s0=d['s0'] * d['cc'],
                                    s1=d['s1'] * d['cc']))
            zsites = list(doubles)
            plain, corr = [], []
            seam = [sg for sg in singles if sg['ws'] is not None]
            nonseam = [sg for sg in singles if sg['ws'] is None]
            for sg in seam:
                if len(zsites) < NZ:
                    # seam single as z-site: cc=0 copy-build, memset patch
                    zsites.append(dict(k=sg['k'], ev=sg['ev'], wa=sg['f0'],
                                       wb=sg['f0'], cc=0.0,
                                       patch=(sg['ws'], 'z', 0.0),
                                       s0=sg['s0'], s1=sg['s1']))
                else:
                    plain.append(sg)
                    corr.append(dict(k=sg['k'], ev=sg['ev'],
                                     col=sg['f0'] + sg['ws'], fo=sg['ws'],
                                     s0=sg['s0'], s1=sg['s1']))
            for sg in nonseam:
                if len(plain) < NSING:
                    plain.append(sg)
                elif len(zsites) < NZ:
                    zsites.append(dict(k=sg['k'], ev=sg['ev'], wa=sg['f0'],
                                       wb=sg['f0'], cc=0.0, patch=None,
                                       s0=sg['s0'], s1=sg['s1']))
                else:
                    raise AssertionError((blk, lh, "slot overflow"))
            assert len(plain) <= NSING, (blk, lh, len(plain))
            assert len(corr) <= NC, (blk, lh, len(corr))
            assert len(plain) >= 1, (blk, lh)   # group-open matmul
            rows.append(dict(z=zsites, sing=plain, corr=corr))
        packs.append(rows)
    return packs


# ----------------------------------------------------------------------------
# device program

def _emit_kernel(tc, aps, E, packs, tgt, spec_ev):
    import concourse.mybir as mybir
    nc = tc.nc
    f16 = mybir.dt.float16
    f32 = mybir.dt.float32
    AL = mybir.AluOpType
    ID = mybir.ActivationFunctionType.Identity

    with tc.tile_pool(name="bigp", bufs=1) as bigp, \
         tc.tile_pool(name="ltp", bufs=6) as ltp, \
         tc.tile_pool(name="zp", bufs=2) as zp, \
         tc.tile_pool(name="zmp", bufs=2) as zmp, \
         tc.tile_pool(name="spzp", bufs=1) as spzp, \
         tc.tile_pool(name="psp", bufs=6, space="PSUM") as psp, \
         tc.tile_pool(name="outp", bufs=10) as outp:

        buf = bigp.tile([128, E * 1024], f16)
        coeft = bigp.tile([128, NSPEC * W], f16)
        biast = bigp.tile([O, 1], f32)
        ltst = bigp.tile([128, NSPEC * O], f16)

        blkv = nc.values_load(aps['blkid'][0:1, 0:1],
                              min_val=0, max_val=3,
                              skip_runtime_bounds_check=True)

        def stage(e):
            base = e * 1024
            src = aps['xb'][e].rearrange("p c w -> (p c) w")
            nc.sync.dma_start(buf[:, base:base + W], src)
            nc.sync.dma_start(buf[:, base + W:base + 2 * W], src)

        def emit_branch(r, ps, zts, zms, ltt, szts, is_spec):
            nmm = len(r['sing']) + len(r['z']) + (NSPEC if is_spec else 0)
            mi = 0
            # first single opens the accumulation group (full width)
            sg = r['sing'][0]
            base = sg['ev'] * 1024
            nc.tensor.matmul(ps, ltt[:, NZ * O:(NZ + 1) * O],
                             buf[:, base + sg['f0']:base + sg['f0'] + W],
                             start=True, stop=(nmm == 1))
            mi = 1
            # 1-col corrections accumulate into the same group
            for j, cr in enumerate(r['corr']):
                si = NZ + NSING + j
                col = cr['ev'] * 1024 + cr['col']
                fo = cr['fo']
                nc.tensor.matmul(ps[:, fo:fo + 1],
                                 ltt[:, si * O:(si + 1) * O],
                                 buf[:, col:col + 1],
                                 start=False, stop=False)
            for j, sg in enumerate(r['sing'][1:], start=1):
                si = NZ + j
                base = sg['ev'] * 1024
                nc.tensor.matmul(ps, ltt[:, si * O:(si + 1) * O],
                                 buf[:, base + sg['f0']:base + sg['f0'] + W],
                                 start=False, stop=(mi == nmm - 1))
                mi += 1
            # z-builds: ts (mult) + tt (add) + patch
            for j, d in enumerate(r['z']):
                base = d['ev'] * 1024
                wina = buf[:, base + d['wa']:base + d['wa'] + W]
                winb = buf[:, base + d['wb']:base + d['wb'] + W]
                if j < NACT:
                    nc.scalar.activation(zms[j], winb, ID, bias=0.0,
                                         scale=float(d['cc']))
                else:
                    nc.vector.tensor_scalar(zms[j], winb, float(d['cc']),
                                            None, AL.mult)
                nc.vector.tensor_tensor(zts[j], zms[j], wina, AL.add)
                if d['patch'] is not None:
                    pc, typ, psc = d['patch']
                    if typ == 'a':
                        nc.vector.tensor_scalar(zts[j][:, pc:pc + 1],
                                                winb[:, pc:pc + 1],
                                                float(psc), None, AL.mult)
                    elif typ == 'b':
                        nc.vector.tensor_copy(zts[j][:, pc:pc + 1],
                                              wina[:, pc:pc + 1])
                    else:
                        nc.vector.memset(zts[j][:, pc:pc + 1], 0.0)
            for j in range(len(r['z'])):
                nc.tensor.matmul(ps, ltt[:, j * O:(j + 1) * O], zts[j],
                                 start=False, stop=(mi == nmm - 1))
                mi += 1
            # antipode specials (band 0, local row 1)
            if is_spec:
                sbase = spec_ev * 1024
                for jj in range(NSPEC):
                    nc.vector.tensor_tensor(
                        szts[jj],
                        buf[:, sbase + 255 + jj:sbase + 255 + jj + W],
                        coeft[:, jj * W:(jj + 1) * W], AL.mult)
                    nc.tensor.matmul(ps, ltst[:, jj * O:(jj + 1) * O],
                                     szts[jj],
                                     start=False, stop=(mi == nmm - 1))
                    mi += 1

        def emit_section(blk):
            staged = 0
            for lh in range(NROW):
                while staged < tgt[lh]:
                    stage(staged)
                    staged += 1
                ltt = ltp.tile([128, LTW], f16, tag="ltt")
                nc.sync.dma_start(ltt, aps['lt'][lh])
                if lh == 0:
                    nc.sync.dma_start(biast, aps['biasd'])
                    nc.sync.dma_start(ltst, aps['lts'])
                    nc.sync.dma_start(coeft, aps['coefr'])
                ps = psp.tile([O, W], f32, tag="ps")
                nz = len(packs[blk][lh]['z'])
                zts = [zp.tile([128, W], f16, tag=f"z{j}", name=f"zt{j}")
                       for j in range(nz)]
                zms = [zmp.tile([128, W], f16, tag=f"zm{j}", name=f"zm{j}")
                       for j in range(nz)]
                is_spec = (blk == 0 and lh == 1)
                szts = [spzp.tile([128, W], f16, tag=f"spz{jj}",
                                  name=f"szt{jj}")
                        for jj in range(NSPEC)] if is_spec else None
                emit_branch(packs[blk][lh], ps, zts, zms, ltt, szts,
                            is_spec=is_spec)
                ot = outp.tile([O, W], f16, tag="out")
                nc.scalar.activation(ot, ps, ID, bias=biast, scale=1.0)
                nc.sync.dma_start(aps['out'][lh], ot)

        for blk in range(4):
            with tc.If(blkv == blk):
                emit_section(blk)


def _get_compiled():
    """Build tables, schedule, and the Bass program once."""
    if 'prog' in _CACHE:
        return _CACHE['prog']
    import concourse.mybir as mybir
    import concourse.tile as tile
    from concourse import bacc

    tt = _build_tap_tables()
    blocks, E, tgt = _build_schedule(tt)
    packs = _pack_rows(tt, blocks)
    spec_ev = int(blocks[0]['need'][1, 1])

    f16 = mybir.dt.float16
    f32 = mybir.dt.float32
    nc = bacc.Bacc("TRN2", target_bir_lowering=False, debug=False,
                   num_devices=NCORES)
    aps = {
        'xb': nc.dram_tensor("xb", [E, 2, C, W], f16,
                             kind="ExternalInput").ap(),
        'lt': nc.dram_tensor("lt", [NROW, 128, LTW], f16,
                             kind="ExternalInput").ap(),
        'lts': nc.dram_tensor("lts", [128, NSPEC * O], f16,
                              kind="ExternalInput").ap(),
        'blkid': nc.dram_tensor("blkid", [1, 1], mybir.dt.int32,
                                kind="ExternalInput").ap(),
        'coefr': nc.dram_tensor("coefr", [128, NSPEC * W], f16,
                                kind="ExternalInput").ap(),
        'biasd': nc.dram_tensor("biasd", [O, 1], f32,
                                kind="ExternalInput").ap(),
        'out': nc.dram_tensor("out", [NROW, O, W], f16,
                              kind="ExternalOutput").ap(),
    }
    with tile.TileContext(nc) as tc:
        _emit_kernel(tc, aps, E, packs, tgt, spec_ev)
    nc.finalize()

    _CACHE['prog'] = (nc, tt, blocks, E, packs)
    return _CACHE['prog']


def _core_inputs(x, weight, bias, tt, blocks, E, packs):
    """Assemble per-core in_maps. Core c = batch (c // 4), band (c % 4)."""
    w3 = weight.reshape(O, C, K).astype(np.float64)
    w2d = np.empty((128, K, O), np.float64)
    w2d[:C] = w3.transpose(1, 2, 0)
    w2d[C:] = w3.transpose(1, 2, 0)
    biasd = np.ascontiguousarray(bias.reshape(O, 1).astype(np.float32))

    lts_on = np.zeros((128, NSPEC * O), np.float16)
    for jj in range(NSPEC):
        lts_on[:C, jj * O:(jj + 1) * O] = w2d[:C, 1, :].astype(np.float16)
    lts_off = np.zeros((128, NSPEC * O), np.float16)

    Gam = tt['Gam'].astype(np.float16)
    coef_on = np.ascontiguousarray(
        np.broadcast_to(Gam[:, None, :], (NSPEC, 128, W))
        .transpose(1, 0, 2).reshape(128, NSPEC * W))
    coef_off = np.zeros((128, NSPEC * W), np.float16)

    lt_blk = []
    for blk in range(4):
        ltv = np.zeros((NROW, 128, LTW), np.float16)
        for lh in range(NROW):
            r = packs[blk][lh]
            for j, d in enumerate(r['z']):
                blkw = np.empty((128, O), np.float64)
                blkw[:64] = w2d[:64, d['k'], :] * d['s0']
                blkw[64:] = w2d[64:, d['k'], :] * d['s1']
                ltv[lh, :, j * O:(j + 1) * O] = blkw.astype(np.float16)
            for j, sg in enumerate(r['sing']):
                si = NZ + j
                blkw = np.empty((128, O), np.float64)
                blkw[:64] = w2d[:64, sg['k'], :] * sg['s0']
                blkw[64:] = w2d[64:, sg['k'], :] * sg['s1']
                ltv[lh, :, si * O:(si + 1) * O] = blkw.astype(np.float16)
            for j, cr in enumerate(r['corr']):
                si = NZ + NSING + j
                blkw = np.empty((128, O), np.float64)
                blkw[:64] = w2d[:64, cr['k'], :] * -cr['s0']
                blkw[64:] = w2d[64:, cr['k'], :] * -cr['s1']
                ltv[lh, :, si * O:(si + 1) * O] = blkw.astype(np.float16)
        lt_blk.append(np.ascontiguousarray(ltv))

    in_maps = []
    for cid in range(NCORES):
        b, blk = cid // 4, cid % 4
        xz = np.concatenate([x[b], np.zeros((C, 1, W), x.dtype)], axis=1)
        xz = xz.astype(np.float16)
        rows = np.asarray(blocks[blk]['events'], np.int64)
        pair_idx = np.stack([rows, rows + 1], axis=1)       # [E, 2]
        xbv = xz[:, pair_idx, :]                            # [C, E, 2, W]
        xbv = np.ascontiguousarray(xbv.transpose(1, 2, 0, 3))  # [E,2,C,W]
        in_maps.append({
            'xb': xbv,
            'lt': lt_blk[blk],
            'lts': lts_on if blk == 0 else lts_off,
            'blkid': np.array([[blk]], np.int32),
            'coefr': coef_on if blk == 0 else coef_off,
            'biasd': biasd,
        })
    return in_maps


def kernel(x, weight, bias):
    from concourse.bass_utils import run_bass_kernel_spmd
    x = np.asarray(x, dtype=np.float32)
    weight = np.asarray(weight, dtype=np.float32)
    bias = np.asarray(bias, dtype=np.float32)

    nc, tt, blocks, E, packs = _get_compiled()
    in_maps = _core_inputs(x, weight, bias, tt, blocks, E, packs)
    res = run_bass_kernel_spmd(nc, in_maps, core_ids=list(range(NCORES)))

    out = np.empty((B, O, H, W), np.float32)
    for cid in range(NCORES):
        b, blk = cid // 4, cid % 4
        oc = res.results[cid]['out'].astype(np.float32)     # [NROW, O, W]
        out[b, :, blk * NROW:(blk + 1) * NROW, :] = oc.transpose(1, 0, 2)
    return out
